# revision 27
# baseline (speedup 1.0000x reference)
"""Trainium2 Bass kernel: bidirectional-LSTM language model (batch-sharded, 8 cores).

Self-contained: hardcodes shapes/sharding for
  S=256, B=32, V=10000, E=32, H=16, 8 NeuronCores.

Math notes (host-folded rescalings):
  sigma(x) = (1 + tanh(x/2)) / 2, so all gate nonlinearities are tanh and the
  whole kernel (recurrence tanh + softmax exp) lives in the single
  `exp_and_others` ACT table set (no table switches).
  Device carries scaled states C = 2c, H = 2h:
    C_t = (t_f+1) c_{t-1} + (t_i+1) g = 0.5*(t_f+1) C_{t-1} + (t_i+1) g
    H_t = (t_o+1) tanh(0.5 C_t)
  with t_* = tanh(z_*/2) for sigmoid gates, g = tanh(z_g); the 1/2 factors are
  folded into the stationary weight matrix on the host.
  log-softmax: logits bounded (|logit| <= 8.25) so no max-shift is needed;
  ln(sum exp) computed with exp-based Newton iterations (no ln table).

Layout constraints honored: SBUF operands must start at partition 0/32/64/96,
DVE ops may have at most one PSUM source. Gate tanh outputs for the sigmoid
gates stay in PSUM (no partition rule there); every 16-row SBUF state tensor
gets its own tile at partition 0.
"""

import os

os.environ.setdefault("MYCRO_LOCAL_CACHE", "1")

import numpy as np

import concourse.bacc as bacc
import concourse.bass as bass
import concourse.tile as tile
from concourse import mybir
from concourse.bass_utils import run_bass_kernel_spmd

# ---------------------------------------------------------------- constants
S, B, V, E, H = 256, 32, 10000, 32, 16
NCORES = 8
BL = B // NCORES          # 4 batch elements per core
COLS = 2 * BL             # 8 recurrence columns: 0..3 LR, 4..7 RL
NSTEP = S - 2             # 254 recurrence steps (t = 0..253)
NBLK = NSTEP + 1          # 255 state blocks (block t = state before step t)
M = S // 2                # 128 output timesteps
KC = E + H + 1            # 49 rows of comb: x, H, ones
KP = 49                   # projection contraction: LR(16) zeros(16) RL(16) ones
NV = 512                  # vocab tile (one PSUM bank of f32)
NT = (V + NV - 1) // NV   # 20 vocab tiles (last one is 272 wide)
VTILES = [(j * NV, min(NV, V - j * NV)) for j in range(NT)]
CH = 32                   # timesteps per projection chunk
NCH = M // CH             # 4 chunks
LN2 = float(np.log(2.0))
# packed-input column offsets: [comb | wall | wsb | c0 | lhsT-init]
C_WALL = NBLK * COLS          # 2040
C_WSB = C_WALL + 128          # 2168
C_C0 = C_WSB + V              # 12168
C_LH = C_C0 + COLS            # 12176
WTOT = C_LH + M               # 12304

f32 = mybir.dt.float32
u32 = mybir.dt.uint32
A = mybir.AluOpType
AF = mybir.ActivationFunctionType
AX = mybir.AxisListType


def _append_dim(ap, step, count):
    """Return a copy of `ap` with an extra innermost free dim [step, count]."""
    pairs = [list(p) for p in ap.ap] + [[step, count]]
    return bass.AP(tensor=ap.tensor, offset=ap.offset, ap=pairs)


def _emit_chunk(nc, c, comb, wsb_sb, lhsT, xsb, sparts, scr_pool, out_pool,
                sm_pool, psum_pool, out_ap):
    """Emit projection for output timesteps [32c, 32c+32)."""
    i0 = CH * c
    # lhsT rows 0..15 <- H_LR: comb H rows, cols 8*(i0+il) + b
    src_lr = comb[E:E + H, COLS * i0: COLS * (i0 + CH)] \
        .rearrange("p (i c) -> p i c", c=COLS)[:, :, 0:BL]
    dst_lr = lhsT[0:H, :].rearrange("p (i b) -> p i b", b=BL)
    nc.gpsimd.tensor_copy(out=dst_lr, in_=src_lr)
    # lhsT rows 32..48 <- H_RL: cols 8*(254 - (i0+il)) + 4 + b (descending)
    hi = COLS * (NSTEP - i0) + BL
    s2 = comb[E:E + H, hi: hi - COLS * CH: -COLS]      # [16, 32] step -8
    src_rl = _append_dim(s2, 1, BL)                    # [16, 32, 4]
    dst_rl = lhsT[32:48, :].rearrange("p (i b) -> p i b", b=BL)
    nc.gpsimd.tensor_copy(out=dst_rl, in_=src_rl)

    # pass A: matmul each vocab tile, exp-accumulate row sums, stash logits
    for j, (n0, nw) in enumerate(VTILES):
        pz = psum_pool.tile([128, NV], f32, tag="projpsum")
        nc.tensor.matmul(pz[:, 0:nw], lhsT[:, :], wsb_sb[:, n0: n0 + nw],
                         start=True, stop=True)
        es = scr_pool.tile([128, NV], f32, tag="expscratch")
        nc.scalar.activation(es[:, 0:nw], pz[:, 0:nw], AF.Exp,
                             accum_out=sparts[:, j:j + 1])
        nc.vector.tensor_copy(out=xsb[:, n0: n0 + nw], in_=pz[:, 0:nw])

    # ln(s) via exponent-seed + 4 Newton iterations (uses only Exp)
    s = sm_pool.tile([128, 1], f32, tag="s")
    nc.vector.reduce_sum(out=s[:, :], in_=sparts[:, :], axis=AX.X)
    sh = sm_pool.tile([128, 1], u32, tag="sh")
    nc.vector.tensor_scalar(sh[:, :], s[:, :].bitcast(u32), 23, None,
                            A.logical_shift_right)
    sh2 = sm_pool.tile([128, 1], u32, tag="sh2")
    nc.vector.tensor_scalar(sh2[:, :], sh[:, :], 0x4B000000, None, A.bitwise_or)
    # y0 = (float(bits>>23 | 0x4B000000) - (2^23 + 126.5)) * ln2
    y = sm_pool.tile([128, 1], f32, tag="y")
    nc.vector.tensor_scalar(y[:, :], sh2[:, :].bitcast(f32),
                            8388608.0 + 126.5, LN2, A.subtract, A.mult)
    for _ in range(4):
        ex = sm_pool.tile([128, 1], f32, tag="nex")
        nc.scalar.activation(ex[:, :], y[:, :], AF.Exp, scale=-1.0)
        uu = sm_pool.tile([128, 1], f32, tag="nuu")
        nc.vector.tensor_scalar(uu[:, :], ex[:, :], s[:, 0:1], None, A.mult)
        nc.vector.scalar_tensor_tensor(y[:, :], y[:, :], 1.0, uu[:, :],
                                       A.subtract, A.add)  # y = (y-1) + s*e^-y
    nln = sm_pool.tile([128, 1], f32, tag="nln")
    nc.vector.tensor_scalar(nln[:, :], y[:, :], -1.0, None, A.mult)

    # pass B: logp = logits - ln(s), stream out
    for n0, nw in VTILES:
        op = out_pool.tile([128, NV], f32, tag="outtile")
        nc.gpsimd.tensor_scalar(op[:, 0:nw], xsb[:, n0: n0 + nw],
                                nln[:, 0:1], None, A.add)
        nc.sync.dma_start(
            out=out_ap[i0:i0 + CH, :, n0: n0 + nw]
            .rearrange("i b n -> (i b) n"),
            in_=op[:, 0:nw])


def _emit(tc, allin, out_ap):
    nc = tc.nc
    with (
        tc.tile_pool(name="persist", bufs=1) as P,
        tc.tile_pool(name="zpsum", bufs=2, space="PSUM") as ZP,
        tc.tile_pool(name="tpsum", bufs=1, space="PSUM") as TPP,
        tc.tile_pool(name="ppsum", bufs=3, space="PSUM") as PP,
        tc.tile_pool(name="scratch", bufs=2) as SC,
        tc.tile_pool(name="outp", bufs=3) as OP,
        tc.tile_pool(name="small", bufs=2) as SM,
    ):
        # one packed input tile; pieces are column slices (single init DMA
        # keeps downstream sync-wait counts within the ISA slot limit)
        ALL = P.tile([KC, WTOT], f32)
        comb = ALL[:, 0:NBLK * COLS]               # x rows / H rows / ones row
        wall_sb = ALL[:, C_WALL:C_WALL + 128]      # gate weights, quad-padded
        wsb_sb = ALL[:, C_WSB:C_WSB + V]           # h2o weights (+bias row)
        ct = ALL[0:H, C_C0:C_C0 + COLS]            # C = 2c (updated in place)
        tif = TPP.tile([64, COLS], f32)            # PSUM: tanh(z_i)@0, t_f@32
        tog = P.tile([64, COLS], f32)              # SBUF: tanh(z_o)@0, g@32
        w1 = P.tile([H, COLS], f32)                # (t_i+1)*g
        w2 = P.tile([H, COLS], f32)                # (t_f+1)*C
        tt = P.tile([H, COLS], f32)                # tanh(c)
        lhsT = ALL[:, C_LH:C_LH + M]               # projection stationary;
        # zero rows 16:32 / ones row 48 come in with the DMA, H rows are
        # rewritten by every chunk's copies.
        xsb = P.tile([128, V], f32)                # chunk logits
        sparts = P.tile([128, NT], f32)            # exp partial sums

        nc.sync.dma_start(out=ALL[:, :], in_=allin)

        chunk_ready = {157: 3, 189: 2, 221: 1}
        for t in range(NSTEP):
            z = ZP.tile([128, COLS], f32, tag="z")
            nc.tensor.matmul(z[:, :], wall_sb[:, :],
                             comb[:, COLS * t: COLS * (t + 1)],
                             start=True, stop=True)
            # tanh halves: i,f -> PSUM (mixed-space stt pairs), o,g -> SBUF
            nc.scalar.activation(tif[:, :], z[0:64, :], AF.Tanh)
            nc.scalar.activation(tog[:, :], z[64:128, :], AF.Tanh)
            nc.vector.scalar_tensor_tensor(w1[:, :], tif[0:16, :], 1.0,
                                           tog[32:48, :], A.add, A.mult)
            nc.vector.scalar_tensor_tensor(w2[:, :], tif[32:48, :], 1.0,
                                           ct[:, :], A.add, A.mult)
            # C = 0.5*(t_f+1)*C + (t_i+1)*g
            nc.vector.scalar_tensor_tensor(ct[:, :], w2[:, :], 0.5,
                                           w1[:, :], A.mult, A.add)
            nc.scalar.activation(tt[:, :], ct[:, :], AF.Tanh, scale=0.5)
            # H_next = (t_o+1)*tanh(c) -> comb H rows of block t+1
            nc.vector.scalar_tensor_tensor(
                comb[E:E + H, COLS * (t + 1): COLS * (t + 2)],
                tog[0:16, :], 1.0, tt[:, :], A.add, A.mult)
            if t in chunk_ready:
                _emit_chunk(nc, chunk_ready[t], comb, wsb_sb, lhsT, xsb,
                            sparts, SC, OP, SM, PP, out_ap)
        _emit_chunk(nc, 0, comb, wsb_sb, lhsT, xsb, sparts, SC, OP, SM, PP,
                    out_ap)


def build_bass():
    nc = bacc.Bacc("TRN2", target_bir_lowering=False, debug=False)
    allin = nc.dram_tensor("allin", [KC, WTOT], f32, kind="ExternalInput")
    out = nc.dram_tensor("out", [M, BL, V], f32, kind="ExternalOutput")
    with tile.TileContext(nc) as tc:
        _emit(tc, allin.ap(), out.ap())
    nc.compile()
    return nc


# ------------------------------------------------------------ host-side prep
def prepare_inputs(inputs):
    """Build the 8 per-core input maps from the full problem inputs."""
    inp = {k: np.asarray(v) for k, v in inputs.items()}
    emb_tab = inp["embedding"].astype(np.float32)
    ib = inp["input_batch"].astype(np.int64)
    emb = emb_tab[ib]                                    # (S, B, E)

    # gate order on device: i, f, o (tanh/2-scaled), then g (=C~, unscaled)
    Wcat = np.concatenate([inp["W_i"], inp["W_f"], inp["W_o"], inp["W_C"]],
                          axis=0).astype(np.float64)     # (64, 48)
    bcat = np.concatenate([inp["b_i"], inp["b_f"], inp["b_o"], inp["b_C"]],
                          axis=0).astype(np.float64)     # (64,)
    rowscale = np.ones(64)
    rowscale[:48] = 0.5                                  # sigmoid-gate rows
    Wp = Wcat * rowscale[:, None]
    Wp[:, E:] *= 0.5                                     # h columns see H = 2h
    bp = bcat * rowscale
    # quadrant-padded stationary: gate m -> columns 32*g + 0:16 (i,f,o,g)
    wall = np.zeros((KC, 128), np.float32)
    for g in range(4):
        cols = slice(32 * g, 32 * g + H)
        rows = slice(H * g, H * (g + 1))
        wall[0:E + H, cols] = Wp[rows].T.astype(np.float32)
        wall[E + H, cols] = bp[rows].astype(np.float32)

    # projection weights: rows 0:16 LR, 16:32 zero, 32:48 RL, 48 bias
    h2o_w = inp["h2o_w"].astype(np.float64)              # (V, 2H)
    wsb = np.zeros((KP, V), np.float32)
    wsb[0:H, :] = (0.5 * h2o_w[:, 0:H].T).astype(np.float32)
    wsb[32:48, :] = (0.5 * h2o_w[:, H:2 * H].T).astype(np.float32)
    wsb[48, :] = inp["h2o_b"].astype(np.float32)

    in_maps = []
    for k in range(NCORES):
        bs = slice(BL * k, BL * (k + 1))
        allin = np.zeros((KC, WTOT), np.float32)
        comb0 = np.zeros((KC, NBLK * COLS), np.float32)
        xs = comb0[0:E].reshape(E, NBLK, COLS)
        xs[:, 0:NSTEP, 0:BL] = emb[0:NSTEP, bs, :].transpose(2, 0, 1)
        xs[:, 0:NSTEP, BL:] = emb[S - 1 - np.arange(NSTEP)][:, bs, :] \
            .transpose(2, 0, 1)
        hs = comb0[E:E + H].reshape(H, NBLK, COLS)
        hs[:, 0, 0:BL] = 2.0 * inp["h0_lr"][bs].T
        hs[:, 0, BL:] = 2.0 * inp["h0_rl"][bs].T
        comb0[E + H, :] = 1.0
        allin[:, 0:NBLK * COLS] = comb0
        allin[:, C_WALL:C_WALL + 128] = wall
        allin[:, C_WSB:C_WSB + V] = wsb
        allin[0:H, C_C0:C_C0 + COLS] = np.concatenate(
            [2.0 * inp["c0_lr"][bs].T, 2.0 * inp["c0_rl"][bs].T], axis=1)
        allin[48, C_LH:C_LH + M] = 1.0   # lhsT ones row (rest stays zero)
        in_maps.append({"allin": allin})
    return in_maps


_CACHE = {}


def get_nc():
    if "nc" not in _CACHE:
        _CACHE["nc"] = build_bass()
    return _CACHE["nc"]


def assemble_output(results):
    preds = np.zeros((S, B, V), np.float32)
    for k in range(NCORES):
        preds[0:M, BL * k: BL * (k + 1), :] = results[k]["out"]
    return preds


def kernel(**inputs):
    in_maps = prepare_inputs(inputs)
    nc = get_nc()
    res = run_bass_kernel_spmd(nc, in_maps, core_ids=list(range(NCORES)))
    return assemble_output(res.results)


# revision 31
# speedup vs baseline: 1.0122x; 1.0122x over previous
"""Trainium2 Bass kernel: bidirectional-LSTM language model (batch-sharded, 8 cores).

Self-contained: hardcodes shapes/sharding for
  S=256, B=32, V=10000, E=32, H=16, 8 NeuronCores.

Math notes (host-folded rescalings):
  sigma(x) = (1 + tanh(x/2)) / 2, so all gate nonlinearities are tanh and the
  whole kernel (recurrence tanh + softmax exp) lives in the single
  `exp_and_others` ACT table set (no table switches).
  Device carries scaled states C = 2c, H = 2h:
    C_t = (t_f+1) c_{t-1} + (t_i+1) g = 0.5*(t_f+1) C_{t-1} + (t_i+1) g
    H_t = (t_o+1) tanh(0.5 C_t)
  with t_* = tanh(z_*/2) for sigmoid gates, g = tanh(z_g); the 1/2 factors are
  folded into the stationary weight matrix on the host.
  log-softmax: logits bounded (|logit| <= 8.25) so no max-shift is needed;
  ln(sum exp) computed with exp-based Newton iterations (no ln table).

Layout constraints honored: SBUF operands must start at partition 0/32/64/96,
DVE ops may have at most one PSUM source. Gate tanh outputs for the sigmoid
gates stay in PSUM (no partition rule there); every 16-row SBUF state tensor
gets its own tile at partition 0.
"""

import os

os.environ.setdefault("MYCRO_LOCAL_CACHE", "1")

import numpy as np

import concourse.bacc as bacc
import concourse.bass as bass
import concourse.tile as tile
from concourse import mybir
from concourse.bass_utils import run_bass_kernel_spmd

# ---------------------------------------------------------------- constants
S, B, V, E, H = 256, 32, 10000, 32, 16
NCORES = 8
BL = B // NCORES          # 4 batch elements per core
COLS = 2 * BL             # 8 recurrence columns: 0..3 LR, 4..7 RL
NSTEP = S - 2             # 254 recurrence steps (t = 0..253)
NBLK = NSTEP + 1          # 255 state blocks (block t = state before step t)
M = S // 2                # 128 output timesteps
KC = E + H + 1            # 49 rows of comb: x, H, ones
KP = 49                   # projection contraction: LR(16) zeros(16) RL(16) ones
NV = 512                  # vocab tile (one PSUM bank of f32)
NT = (V + NV - 1) // NV   # 20 vocab tiles (last one is 272 wide)
VTILES = [(j * NV, min(NV, V - j * NV)) for j in range(NT)]
CH = 32                   # timesteps per projection chunk
NCH = M // CH             # 4 chunks
LN2 = float(np.log(2.0))
# packed-input column offsets: [comb | wall | wsb | c0 | lhsT-init]
C_WALL = NBLK * COLS          # 2040
C_WSB = C_WALL + 128          # 2168
C_C0 = C_WSB + V              # 12168
C_LH = C_C0 + COLS            # 12176
WTOT = C_LH + M               # 12304

f32 = mybir.dt.float32
u32 = mybir.dt.uint32
A = mybir.AluOpType
AF = mybir.ActivationFunctionType
AX = mybir.AxisListType


def _append_dim(ap, step, count):
    """Return a copy of `ap` with an extra innermost free dim [step, count]."""
    pairs = [list(p) for p in ap.ap] + [[step, count]]
    return bass.AP(tensor=ap.tensor, offset=ap.offset, ap=pairs)


def _chunk_units(nc, c, comb, wsb_sb, lhsT, xsb, sparts, scr_pool, out_pool,
                 sm_pool, psum_pool, out_ap):
    """Yield projection work-unit closures for chunk c. Units are emitted
    between recurrence steps so long projection instructions don't
    head-of-line-block the recurrence chain on any engine."""
    i0 = CH * c

    def u_copies():
        # lhsT rows 0..15 <- H_LR: comb H rows, cols 8*(i0+il) + b
        src_lr = comb[E:E + H, COLS * i0: COLS * (i0 + CH)] \
            .rearrange("p (i c) -> p i c", c=COLS)[:, :, 0:BL]
        dst_lr = lhsT[0:H, :].rearrange("p (i b) -> p i b", b=BL)
        nc.gpsimd.tensor_copy(out=dst_lr, in_=src_lr)
        # lhsT rows 32..48 <- H_RL: cols 8*(254-(i0+il)) + 4 + b (descending)
        hi = COLS * (NSTEP - i0) + BL
        s2 = comb[E:E + H, hi: hi - COLS * CH: -COLS]      # [16, 32] step -8
        src_rl = _append_dim(s2, 1, BL)                    # [16, 32, 4]
        dst_rl = lhsT[32:48, :].rearrange("p (i b) -> p i b", b=BL)
        nc.gpsimd.tensor_copy(out=dst_rl, in_=src_rl)
    yield u_copies

    def u_tile(j, n0, nw):
        def f():
            pz = psum_pool.tile([128, NV], f32, tag="projpsum")
            nc.tensor.matmul(pz[:, 0:nw], lhsT[:, :], wsb_sb[:, n0: n0 + nw],
                             start=True, stop=True)
            es = scr_pool.tile([128, NV], f32, tag="expscratch")
            nc.scalar.activation(es[:, 0:nw], pz[:, 0:nw], AF.Exp,
                                 accum_out=sparts[:, j:j + 1])
            nc.vector.tensor_copy(out=xsb[:, n0: n0 + nw], in_=pz[:, 0:nw])
        return f
    for j, (n0, nw) in enumerate(VTILES):
        yield u_tile(j, n0, nw)

    nln = sm_pool.tile([128, 1], f32, tag="nln")

    def u_newton():
        # ln(s) via exponent-seed + 4 Newton iterations (uses only Exp)
        s = sm_pool.tile([128, 1], f32, tag="s")
        nc.vector.reduce_sum(out=s[:, :], in_=sparts[:, :], axis=AX.X)
        sh = sm_pool.tile([128, 1], u32, tag="sh")
        nc.vector.tensor_scalar(sh[:, :], s[:, :].bitcast(u32), 23, None,
                                A.logical_shift_right)
        sh2 = sm_pool.tile([128, 1], u32, tag="sh2")
        nc.vector.tensor_scalar(sh2[:, :], sh[:, :], 0x4B000000, None,
                                A.bitwise_or)
        # y0 = (float(bits>>23 | 0x4B000000) - (2^23 + 126.5)) * ln2
        y = sm_pool.tile([128, 1], f32, tag="y")
        nc.vector.tensor_scalar(y[:, :], sh2[:, :].bitcast(f32),
                                8388608.0 + 126.5, LN2, A.subtract, A.mult)
        for _ in range(4):
            ex = sm_pool.tile([128, 1], f32, tag="nex")
            nc.scalar.activation(ex[:, :], y[:, :], AF.Exp, scale=-1.0)
            uu = sm_pool.tile([128, 1], f32, tag="nuu")
            nc.vector.tensor_scalar(uu[:, :], ex[:, :], s[:, 0:1], None,
                                    A.mult)
            nc.vector.scalar_tensor_tensor(y[:, :], y[:, :], 1.0, uu[:, :],
                                           A.subtract, A.add)
        nc.vector.tensor_scalar(nln[:, :], y[:, :], -1.0, None, A.mult)
    yield u_newton

    def u_out(n0, nw):
        def f():
            op = out_pool.tile([128, NV], f32, tag="outtile")
            nc.gpsimd.tensor_scalar(op[:, 0:nw], xsb[:, n0: n0 + nw],
                                    nln[:, 0:1], None, A.add)
            nc.sync.dma_start(
                out=out_ap[i0:i0 + CH, :, n0: n0 + nw]
                .rearrange("i b n -> (i b) n"),
                in_=op[:, 0:nw])
        return f
    for n0, nw in VTILES:
        yield u_out(n0, nw)


def _emit(tc, allin, out_ap):
    nc = tc.nc
    with (
        tc.tile_pool(name="persist", bufs=1) as P,
        tc.tile_pool(name="zpsum", bufs=2, space="PSUM") as ZP,
        tc.tile_pool(name="tpsum", bufs=1, space="PSUM") as TPP,
        tc.tile_pool(name="ppsum", bufs=3, space="PSUM") as PP,
        tc.tile_pool(name="scratch", bufs=2) as SC,
        tc.tile_pool(name="outp", bufs=3) as OP,
        tc.tile_pool(name="small", bufs=2) as SM,
    ):
        # one packed input tile; pieces are column slices (single init DMA
        # keeps downstream sync-wait counts within the ISA slot limit)
        ALL = P.tile([KC, WTOT], f32)
        comb = ALL[:, 0:NBLK * COLS]               # x rows / H rows / ones row
        wall_sb = ALL[:, C_WALL:C_WALL + 128]      # gate weights, quad-padded
        wsb_sb = ALL[:, C_WSB:C_WSB + V]           # h2o weights (+bias row)
        ct = ALL[0:H, C_C0:C_C0 + COLS]            # C = 2c (updated in place)
        tif = TPP.tile([64, COLS], f32)            # PSUM: tanh(z_i)@0, t_f@32
        tog = P.tile([64, COLS], f32)              # SBUF: tanh(z_o)@0, g@32
        w1 = P.tile([H, COLS], f32)                # (t_i+1)*g
        w2 = P.tile([H, COLS], f32)                # (t_f+1)*C
        tt = P.tile([H, COLS], f32)                # tanh(c)
        lhsT = ALL[:, C_LH:C_LH + M]               # projection stationary;
        # zero rows 16:32 / ones row 48 come in with the DMA, H rows are
        # rewritten by every chunk's copies.
        xsb = P.tile([128, V], f32)                # chunk logits
        sparts = P.tile([128, NT], f32)            # exp partial sums

        nc.sync.dma_start(out=ALL[:, :], in_=allin)

        chunk_ready = {157: 3, 189: 2, 221: 1}
        pending = []
        for t in range(NSTEP):
            z = ZP.tile([128, COLS], f32, tag="z")
            nc.tensor.matmul(z[:, :], wall_sb[:, :],
                             comb[:, COLS * t: COLS * (t + 1)],
                             start=True, stop=True)
            # tanh halves: i,f -> PSUM (mixed-space stt pairs), o,g -> SBUF
            nc.scalar.activation(tif[:, :], z[0:64, :], AF.Tanh)
            nc.scalar.activation(tog[:, :], z[64:128, :], AF.Tanh)
            nc.vector.scalar_tensor_tensor(w1[:, :], tif[0:16, :], 1.0,
                                           tog[32:48, :], A.add, A.mult)
            nc.vector.scalar_tensor_tensor(w2[:, :], tif[32:48, :], 1.0,
                                           ct[:, :], A.add, A.mult)
            # C = 0.5*(t_f+1)*C + (t_i+1)*g
            nc.vector.scalar_tensor_tensor(ct[:, :], w2[:, :], 0.5,
                                           w1[:, :], A.mult, A.add)
            nc.scalar.activation(tt[:, :], ct[:, :], AF.Tanh, scale=0.5)
            # H_next = (t_o+1)*tanh(c) -> comb H rows of block t+1
            nc.vector.scalar_tensor_tensor(
                comb[E:E + H, COLS * (t + 1): COLS * (t + 2)],
                tog[0:16, :], 1.0, tt[:, :], A.add, A.mult)
            if t in chunk_ready:
                pending.extend(_chunk_units(nc, chunk_ready[t], comb, wsb_sb,
                                            lhsT, xsb, sparts, SC, OP, SM,
                                            PP, out_ap))
            for fn in pending[:2]:
                fn()
            del pending[:2]
        for fn in pending:
            fn()
        for fn in _chunk_units(nc, 0, comb, wsb_sb, lhsT, xsb, sparts, SC,
                               OP, SM, PP, out_ap):
            fn()


def build_bass():
    nc = bacc.Bacc("TRN2", target_bir_lowering=False, debug=False)
    allin = nc.dram_tensor("allin", [KC, WTOT], f32, kind="ExternalInput")
    out = nc.dram_tensor("out", [M, BL, V], f32, kind="ExternalOutput")
    with tile.TileContext(nc) as tc:
        _emit(tc, allin.ap(), out.ap())
    nc.compile()
    return nc


# ------------------------------------------------------------ host-side prep
def prepare_inputs(inputs):
    """Build the 8 per-core input maps from the full problem inputs."""
    inp = {k: np.asarray(v) for k, v in inputs.items()}
    emb_tab = inp["embedding"].astype(np.float32)
    ib = inp["input_batch"].astype(np.int64)
    emb = emb_tab[ib]                                    # (S, B, E)

    # gate order on device: i, f, o (tanh/2-scaled), then g (=C~, unscaled)
    Wcat = np.concatenate([inp["W_i"], inp["W_f"], inp["W_o"], inp["W_C"]],
                          axis=0).astype(np.float64)     # (64, 48)
    bcat = np.concatenate([inp["b_i"], inp["b_f"], inp["b_o"], inp["b_C"]],
                          axis=0).astype(np.float64)     # (64,)
    rowscale = np.ones(64)
    rowscale[:48] = 0.5                                  # sigmoid-gate rows
    Wp = Wcat * rowscale[:, None]
    Wp[:, E:] *= 0.5                                     # h columns see H = 2h
    bp = bcat * rowscale
    # quadrant-padded stationary: gate m -> columns 32*g + 0:16 (i,f,o,g)
    wall = np.zeros((KC, 128), np.float32)
    for g in range(4):
        cols = slice(32 * g, 32 * g + H)
        rows = slice(H * g, H * (g + 1))
        wall[0:E + H, cols] = Wp[rows].T.astype(np.float32)
        wall[E + H, cols] = bp[rows].astype(np.float32)

    # projection weights: rows 0:16 LR, 16:32 zero, 32:48 RL, 48 bias
    h2o_w = inp["h2o_w"].astype(np.float64)              # (V, 2H)
    wsb = np.zeros((KP, V), np.float32)
    wsb[0:H, :] = (0.5 * h2o_w[:, 0:H].T).astype(np.float32)
    wsb[32:48, :] = (0.5 * h2o_w[:, H:2 * H].T).astype(np.float32)
    wsb[48, :] = inp["h2o_b"].astype(np.float32)

    in_maps = []
    for k in range(NCORES):
        bs = slice(BL * k, BL * (k + 1))
        allin = np.zeros((KC, WTOT), np.float32)
        comb0 = np.zeros((KC, NBLK * COLS), np.float32)
        xs = comb0[0:E].reshape(E, NBLK, COLS)
        xs[:, 0:NSTEP, 0:BL] = emb[0:NSTEP, bs, :].transpose(2, 0, 1)
        xs[:, 0:NSTEP, BL:] = emb[S - 1 - np.arange(NSTEP)][:, bs, :] \
            .transpose(2, 0, 1)
        hs = comb0[E:E + H].reshape(H, NBLK, COLS)
        hs[:, 0, 0:BL] = 2.0 * inp["h0_lr"][bs].T
        hs[:, 0, BL:] = 2.0 * inp["h0_rl"][bs].T
        comb0[E + H, :] = 1.0
        allin[:, 0:NBLK * COLS] = comb0
        allin[:, C_WALL:C_WALL + 128] = wall
        allin[:, C_WSB:C_WSB + V] = wsb
        allin[0:H, C_C0:C_C0 + COLS] = np.concatenate(
            [2.0 * inp["c0_lr"][bs].T, 2.0 * inp["c0_rl"][bs].T], axis=1)
        allin[48, C_LH:C_LH + M] = 1.0   # lhsT ones row (rest stays zero)
        in_maps.append({"allin": allin})
    return in_maps


_CACHE = {}


def get_nc():
    if "nc" not in _CACHE:
        _CACHE["nc"] = build_bass()
    return _CACHE["nc"]


def assemble_output(results):
    preds = np.zeros((S, B, V), np.float32)
    for k in range(NCORES):
        preds[0:M, BL * k: BL * (k + 1), :] = results[k]["out"]
    return preds


def kernel(**inputs):
    in_maps = prepare_inputs(inputs)
    nc = get_nc()
    res = run_bass_kernel_spmd(nc, in_maps, core_ids=list(range(NCORES)))
    return assemble_output(res.results)


# revision 38
# speedup vs baseline: 1.0480x; 1.0354x over previous
"""Trainium2 Bass kernel: bidirectional-LSTM language model (batch-sharded, 8 cores).

Self-contained: hardcodes shapes/sharding for
  S=256, B=32, V=10000, E=32, H=16, 8 NeuronCores.

Math notes (host-folded rescalings):
  sigma(x) = (1 + tanh(x/2)) / 2, so all gate nonlinearities are tanh and the
  whole kernel (recurrence tanh + softmax exp) lives in the single
  `exp_and_others` ACT table set (no table switches).
  Device carries scaled states C = 2c, H = 2h:
    C_t = (t_f+1) c_{t-1} + (t_i+1) g = 0.5*(t_f+1) C_{t-1} + (t_i+1) g
    H_t = (t_o+1) tanh(0.5 C_t)
  with t_* = tanh(z_*/2) for sigmoid gates, g = tanh(z_g); the 1/2 factors are
  folded into the stationary weight matrix on the host.
  log-softmax: logits bounded (|logit| <= 8.25) so no max-shift is needed;
  ln(sum exp) computed with exp-based Newton iterations (no ln table).

Layout constraints honored: SBUF operands must start at partition 0/32/64/96,
DVE ops may have at most one PSUM source. Gate tanh outputs for the sigmoid
gates stay in PSUM (no partition rule there); every 16-row SBUF state tensor
gets its own tile at partition 0.
"""

import os

os.environ.setdefault("MYCRO_LOCAL_CACHE", "1")

import numpy as np

import concourse.bacc as bacc
import concourse.bass as bass
import concourse.tile as tile
from concourse import mybir
from concourse.bass_utils import run_bass_kernel_spmd

# ---------------------------------------------------------------- constants
S, B, V, E, H = 256, 32, 10000, 32, 16
NCORES = 8
BL = B // NCORES          # 4 batch elements per core
COLS = 2 * BL             # 8 recurrence columns: 0..3 LR, 4..7 RL
NSTEP = S - 2             # 254 recurrence steps (t = 0..253)
NBLK = NSTEP + 1          # 255 state blocks (block t = state before step t)
M = S // 2                # 128 output timesteps
KC = E + H + 1            # 49 rows of comb: x, H, ones
KP = 49                   # projection contraction: LR(16) zeros(16) RL(16) ones
NV = 512                  # vocab tile (one PSUM bank of f32)
HNV = NV // 2             # half-tile instruction granularity
NT = (V + NV - 1) // NV   # 20 vocab tiles (last one is 272 wide)
VTILES = [(j * NV, min(NV, V - j * NV)) for j in range(NT)]
CH = 32                   # timesteps per projection chunk
NCH = M // CH             # 4 chunks
LN2 = float(np.log(2.0))
# packed-input column offsets: [comb | wall | c0 | lhsT-init | wsb].
# wsb (40KB/partition) sits last and loads via a second DMA so step 0 only
# waits for the small head (~9KB/partition).
C_WALL = NBLK * COLS          # 2040
C_C0 = C_WALL + 128           # 2168
C_LH = C_C0 + COLS            # 2176
C_WSB = C_LH + M              # 2304
WTOT = C_WSB + V              # 12304

f32 = mybir.dt.float32
u32 = mybir.dt.uint32
A = mybir.AluOpType
AF = mybir.ActivationFunctionType
AX = mybir.AxisListType


def _append_dim(ap, step, count):
    """Return a copy of `ap` with an extra innermost free dim [step, count]."""
    pairs = [list(p) for p in ap.ap] + [[step, count]]
    return bass.AP(tensor=ap.tensor, offset=ap.offset, ap=pairs)


def _chunk_units(nc, c, comb, wsb_sb, lhsT, xsb, sparts, scr_pool, out_pool,
                 sm_pool, psum_pool, out_ap):
    """Yield projection work-unit closures for chunk c. Units are emitted
    between recurrence steps so long projection instructions don't
    head-of-line-block the recurrence chain on any engine."""
    i0 = CH * c

    def u_copies():
        # lhsT rows 0..15 <- H_LR: comb H rows, cols 8*(i0+il) + b
        src_lr = comb[E:E + H, COLS * i0: COLS * (i0 + CH)] \
            .rearrange("p (i c) -> p i c", c=COLS)[:, :, 0:BL]
        dst_lr = lhsT[0:H, :].rearrange("p (i b) -> p i b", b=BL)
        nc.gpsimd.tensor_copy(out=dst_lr, in_=src_lr)
        # lhsT rows 32..48 <- H_RL: cols 8*(254-(i0+il)) + 4 + b (descending)
        hi = COLS * (NSTEP - i0) + BL
        s2 = comb[E:E + H, hi: hi - COLS * CH: -COLS]      # [16, 32] step -8
        src_rl = _append_dim(s2, 1, BL)                    # [16, 32, 4]
        dst_rl = lhsT[32:48, :].rearrange("p (i b) -> p i b", b=BL)
        nc.gpsimd.tensor_copy(out=dst_rl, in_=src_rl)
    yield u_copies

    def u_tile(j, n0, nw):
        def f():
            pz = psum_pool.tile([128, NV], f32, tag="projpsum")
            nc.tensor.matmul(pz[:, 0:nw], lhsT[:, :], wsb_sb[:, n0: n0 + nw],
                             start=True, stop=True)
            es = scr_pool.tile([128, NV], f32, tag="expscratch")
            nc.scalar.activation(es[:, 0:nw], pz[:, 0:nw], AF.Exp,
                                 accum_out=sparts[:, j:j + 1])
            nc.vector.tensor_copy(out=xsb[:, n0: n0 + nw], in_=pz[:, 0:nw])
        return f
    for j, (n0, nw) in enumerate(VTILES):
        yield u_tile(j, n0, nw)

    nln = sm_pool.tile([128, 1], f32, tag="nln")

    def u_newton():
        # ln(s) via exponent-seed + 4 Newton iterations (uses only Exp)
        s = sm_pool.tile([128, 1], f32, tag="s")
        nc.vector.reduce_sum(out=s[:, :], in_=sparts[:, :], axis=AX.X)
        sh = sm_pool.tile([128, 1], u32, tag="sh")
        nc.vector.tensor_scalar(sh[:, :], s[:, :].bitcast(u32), 23, None,
                                A.logical_shift_right)
        sh2 = sm_pool.tile([128, 1], u32, tag="sh2")
        nc.vector.tensor_scalar(sh2[:, :], sh[:, :], 0x4B000000, None,
                                A.bitwise_or)
        # y0 = (float(bits>>23 | 0x4B000000) - (2^23 + 126.5)) * ln2
        y = sm_pool.tile([128, 1], f32, tag="y")
        nc.vector.tensor_scalar(y[:, :], sh2[:, :].bitcast(f32),
                                8388608.0 + 126.5, LN2, A.subtract, A.mult)
        for _ in range(4):
            ex = sm_pool.tile([128, 1], f32, tag="nex")
            nc.scalar.activation(ex[:, :], y[:, :], AF.Exp, scale=-1.0)
            uu = sm_pool.tile([128, 1], f32, tag="nuu")
            nc.vector.tensor_scalar(uu[:, :], ex[:, :], s[:, 0:1], None,
                                    A.mult)
            nc.vector.scalar_tensor_tensor(y[:, :], y[:, :], 1.0, uu[:, :],
                                           A.subtract, A.add)
        nc.vector.tensor_scalar(nln[:, :], y[:, :], -1.0, None, A.mult)
    yield u_newton

    def u_out(n0, nw):
        def f():
            op = out_pool.tile([128, NV], f32, tag="outtile")
            nc.gpsimd.tensor_scalar(op[:, 0:nw], xsb[:, n0: n0 + nw],
                                    nln[:, 0:1], None, A.add)
            nc.sync.dma_start(
                out=out_ap[i0:i0 + CH, :, n0: n0 + nw]
                .rearrange("i b n -> (i b) n"),
                in_=op[:, 0:nw])
        return f
    for n0, nw in VTILES:
        yield u_out(n0, nw)


def _emit(tc, allin, out_ap):
    nc = tc.nc
    with (
        tc.tile_pool(name="persist", bufs=1) as P,
        tc.tile_pool(name="zpsum", bufs=2, space="PSUM") as ZP,
        tc.tile_pool(name="tpsum", bufs=1, space="PSUM") as TPP,
        tc.tile_pool(name="ppsum", bufs=3, space="PSUM") as PP,
        tc.tile_pool(name="scratch", bufs=2) as SC,
        tc.tile_pool(name="outp", bufs=3) as OP,
        tc.tile_pool(name="small", bufs=2) as SM,
    ):
        # one packed input tile; pieces are column slices (single init DMA
        # keeps downstream sync-wait counts within the ISA slot limit)
        ALL = P.tile([KC, WTOT], f32)
        comb = ALL[:, 0:NBLK * COLS]               # x rows / H rows / ones row
        wall_sb = ALL[:, C_WALL:C_WALL + 128]      # gate weights, quad-padded
        wsb_sb = ALL[:, C_WSB:C_WSB + V]           # h2o weights (+bias row)
        ct = ALL[0:H, C_C0:C_C0 + COLS]            # C = 2c (updated in place)
        tif = TPP.tile([64, COLS], f32)            # PSUM: tanh(z_i)@0, t_f@32
        tog = P.tile([64, COLS], f32)              # SBUF: tanh(z_o)@0, g@32
        w1 = P.tile([H, COLS], f32)                # (t_i+1)*g
        w2 = P.tile([H, COLS], f32)                # (t_f+1)*C
        tt = P.tile([H, COLS], f32)                # tanh(c)
        lhsT = ALL[:, C_LH:C_LH + M]               # projection stationary;
        # zero rows 16:32 / ones row 48 come in with the DMA, H rows are
        # rewritten by every chunk's copies.
        xsb = P.tile([128, V], f32)                # chunk logits
        sparts = P.tile([128, NT], f32)            # exp partial sums

        nc.sync.dma_start(out=ALL[:, 0:C_WSB], in_=allin[:, 0:C_WSB])
        nc.sync.dma_start(out=ALL[:, C_WSB:WTOT], in_=allin[:, C_WSB:WTOT])

        chunk_ready = {157: 3, 189: 2, 221: 1}
        pending = []
        for t in range(NSTEP):
            z = ZP.tile([128, COLS], f32, tag="z")
            nc.tensor.matmul(z[:, :], wall_sb[:, :],
                             comb[:, COLS * t: COLS * (t + 1)],
                             start=True, stop=True)
            # tanh halves: i,f -> PSUM (mixed-space stt pairs), o,g -> SBUF
            nc.scalar.activation(tif[:, :], z[0:64, :], AF.Tanh)
            nc.scalar.activation(tog[:, :], z[64:128, :], AF.Tanh)
            nc.vector.scalar_tensor_tensor(w1[:, :], tif[0:16, :], 1.0,
                                           tog[32:48, :], A.add, A.mult)
            nc.vector.scalar_tensor_tensor(w2[:, :], tif[32:48, :], 1.0,
                                           ct[:, :], A.add, A.mult)
            # C = 0.5*(t_f+1)*C + (t_i+1)*g
            nc.vector.scalar_tensor_tensor(ct[:, :], w2[:, :], 0.5,
                                           w1[:, :], A.mult, A.add)
            nc.scalar.activation(tt[:, :], ct[:, :], AF.Tanh, scale=0.5)
            # H_next = (t_o+1)*tanh(c) -> comb H rows of block t+1
            nc.vector.scalar_tensor_tensor(
                comb[E:E + H, COLS * (t + 1): COLS * (t + 2)],
                tog[0:16, :], 1.0, tt[:, :], A.add, A.mult)
            if t in chunk_ready:
                pending.extend(_chunk_units(nc, chunk_ready[t], comb, wsb_sb,
                                            lhsT, xsb, sparts, SC, OP, SM,
                                            PP, out_ap))
            for fn in pending[:2]:
                fn()
            del pending[:2]
        for fn in pending:
            fn()
        for fn in _chunk_units(nc, 0, comb, wsb_sb, lhsT, xsb, sparts, SC,
                               OP, SM, PP, out_ap):
            fn()


def build_bass():
    nc = bacc.Bacc("TRN2", target_bir_lowering=False, debug=False)
    allin = nc.dram_tensor("allin", [KC, WTOT], f32, kind="ExternalInput")
    out = nc.dram_tensor("out", [M, BL, V], f32, kind="ExternalOutput")
    with tile.TileContext(nc) as tc:
        _emit(tc, allin.ap(), out.ap())
    nc.compile()
    return nc


# ------------------------------------------------------------ host-side prep
def prepare_inputs(inputs):
    """Build the 8 per-core input maps from the full problem inputs."""
    inp = {k: np.asarray(v) for k, v in inputs.items()}
    emb_tab = inp["embedding"].astype(np.float32)
    ib = inp["input_batch"].astype(np.int64)
    emb = emb_tab[ib]                                    # (S, B, E)

    # gate order on device: i, f, o (tanh/2-scaled), then g (=C~, unscaled)
    Wcat = np.concatenate([inp["W_i"], inp["W_f"], inp["W_o"], inp["W_C"]],
                          axis=0).astype(np.float64)     # (64, 48)
    bcat = np.concatenate([inp["b_i"], inp["b_f"], inp["b_o"], inp["b_C"]],
                          axis=0).astype(np.float64)     # (64,)
    rowscale = np.ones(64)
    rowscale[:48] = 0.5                                  # sigmoid-gate rows
    Wp = Wcat * rowscale[:, None]
    Wp[:, E:] *= 0.5                                     # h columns see H = 2h
    bp = bcat * rowscale
    # quadrant-padded stationary: gate m -> columns 32*g + 0:16 (i,f,o,g)
    wall = np.zeros((KC, 128), np.float32)
    for g in range(4):
        cols = slice(32 * g, 32 * g + H)
        rows = slice(H * g, H * (g + 1))
        wall[0:E + H, cols] = Wp[rows].T.astype(np.float32)
        wall[E + H, cols] = bp[rows].astype(np.float32)

    # projection weights: rows 0:16 LR, 16:32 zero, 32:48 RL, 48 bias
    h2o_w = inp["h2o_w"].astype(np.float64)              # (V, 2H)
    wsb = np.zeros((KP, V), np.float32)
    wsb[0:H, :] = (0.5 * h2o_w[:, 0:H].T).astype(np.float32)
    wsb[32:48, :] = (0.5 * h2o_w[:, H:2 * H].T).astype(np.float32)
    wsb[48, :] = inp["h2o_b"].astype(np.float32)

    in_maps = []
    for k in range(NCORES):
        bs = slice(BL * k, BL * (k + 1))
        allin = np.zeros((KC, WTOT), np.float32)
        comb0 = np.zeros((KC, NBLK * COLS), np.float32)
        xs = comb0[0:E].reshape(E, NBLK, COLS)
        xs[:, 0:NSTEP, 0:BL] = emb[0:NSTEP, bs, :].transpose(2, 0, 1)
        xs[:, 0:NSTEP, BL:] = emb[S - 1 - np.arange(NSTEP)][:, bs, :] \
            .transpose(2, 0, 1)
        hs = comb0[E:E + H].reshape(H, NBLK, COLS)
        hs[:, 0, 0:BL] = 2.0 * inp["h0_lr"][bs].T
        hs[:, 0, BL:] = 2.0 * inp["h0_rl"][bs].T
        comb0[E + H, :] = 1.0
        allin[:, 0:NBLK * COLS] = comb0
        allin[:, C_WALL:C_WALL + 128] = wall
        allin[:, C_WSB:C_WSB + V] = wsb
        allin[0:H, C_C0:C_C0 + COLS] = np.concatenate(
            [2.0 * inp["c0_lr"][bs].T, 2.0 * inp["c0_rl"][bs].T], axis=1)
        allin[48, C_LH:C_LH + M] = 1.0   # lhsT ones row (rest stays zero)
        in_maps.append({"allin": allin})
    return in_maps


_CACHE = {}


def get_nc():
    if "nc" not in _CACHE:
        _CACHE["nc"] = build_bass()
    return _CACHE["nc"]


def assemble_output(results):
    preds = np.zeros((S, B, V), np.float32)
    for k in range(NCORES):
        preds[0:M, BL * k: BL * (k + 1), :] = results[k]["out"]
    return preds


def kernel(**inputs):
    in_maps = prepare_inputs(inputs)
    nc = get_nc()
    res = run_bass_kernel_spmd(nc, in_maps, core_ids=list(range(NCORES)))
    return assemble_output(res.results)
